# revision 42
# baseline (speedup 1.0000x reference)
"""Trainium2 Bass kernel for a single-head attention + FFN transformer block.

Math (per batch b), computed in bf16 matmuls with fp32 PSUM accumulation:
  S^T  = K @ Qs^T                  Qs = Q/sqrt(d), invalid-q rows zeroed (host)
  S^T += kbias[k] (x) qmask[q]     rank-1 matmul; kbias = 0 valid / -3e4 masked
  E^T  = exp(S^T)                  invalid-q columns become exactly 1.0, so the
                                   softmax degenerates to uniform over all 512
                                   keys, matching the reference's fully-masked
                                   row behaviour.
  rowsum^T[q] = sum_k E^T[k,q]     tiny N=1 matmuls vs a ones column
  att  = (E^T' @ V) * recip        E^T used as stationary, [q,d] output
  y    = LN1(Q + att)
  H^T  = relu(W1^T . y^T)          y^T via PE transposes
  Z    = H^T' @ W2^T
  out  = LN2(y + Z)                (+b2 shift cancels inside LN2 exactly)

Software pipeline (PE program order per steady-state batch):
  rowsumT(b) | U(b) | S(b+1) kt-groups interleaved with yT(b) | FFN1(b) | FFN2(b)
so each yT transpose group arrives only after its qt's LN1 chain has
cleared the DVE queue; the last batch staggers yT inside U and splits
FFN1 into two half-q passes instead. att-scale + residual-add are fused
into one DVE scalar_tensor_tensor; exps interleave between the yT
groups (copies are ACT-table-neutral) so the Exp/Sqrt activation tables
swap only twice per batch; y and the transposes are bf16 (bf16 PE
transposes stream at 1 col/cycle vs 2 for fp32r).

All HBM inputs are host-swizzled partition-major so every DMA descriptor
covers 4-16KB of contiguous HBM per partition; batch 0 computes S^T
dt-outer (borrowing idle psU banks) so its matmuls chase the arriving
chunks, with a short bf16 identity warmup to lift the PE HAM clock gate
before data lands.

Sharding: pure data-parallel, 4 batches per core across 8 NeuronCores.
"""

import sys

sys.path.insert(0, "/opt/trn_rl_repo")

import numpy as np
import ml_dtypes

import concourse.bass as bass
import concourse.bacc as bacc
import concourse.mybir as mybir
from concourse import tile
from concourse.bass_utils import run_bass_kernel_spmd

B, QTL, KTL, D = 32, 512, 512, 1024
NCORES = 8
BL = B // NCORES  # batches per core
P = 128
NQT = QTL // P  # 4 q tiles
NKT = KTL // P  # 4 k tiles
NDT = D // P    # 8 d tiles
NCH = D // 512  # 2 free-dim chunks of 512
EPS = 1e-5
NEG = -30000.0
NWARM = 80

F32 = mybir.dt.float32
F32R = mybir.dt.float32r
I32 = mybir.dt.int32
BF16 = mybir.dt.bfloat16
AF = mybir.ActivationFunctionType
ALU = mybir.AluOpType
BF16NP = ml_dtypes.bfloat16


def _build(apply1: bool, apply2: bool, b1f: float):
    nc = bacc.Bacc(None, target_bir_lowering=False)

    QTd = nc.dram_tensor("QTp", [BL, P, NDT, QTL], BF16, kind="ExternalInput")
    KTd = nc.dram_tensor("KTp", [BL, P, NDT, KTL], BF16, kind="ExternalInput")
    Vd = nc.dram_tensor("Vp", [BL, P, NKT, D], BF16, kind="ExternalInput")
    Qd = nc.dram_tensor("Qp", [BL, P, NQT, D], F32, kind="ExternalInput")
    W1Td = nc.dram_tensor("W1Tp", [P, NDT, D], BF16, kind="ExternalInput")
    W2Td = nc.dram_tensor("W2Tp", [P, NDT, D], BF16, kind="ExternalInput")
    QMd = nc.dram_tensor("QMp", [BL, QTL], BF16, kind="ExternalInput")
    KBd = nc.dram_tensor("KBp", [BL, KTL], BF16, kind="ExternalInput")
    IDd = nc.dram_tensor("IDp", [P, P], F32, kind="ExternalInput")
    IDBd = nc.dram_tensor("IDBp", [P, P], BF16, kind="ExternalInput")
    ONEd = nc.dram_tensor("ONEp", [P, 1], BF16, kind="ExternalInput")
    if apply1:
        G1d = nc.dram_tensor("G1p", [D], F32, kind="ExternalInput")
        B1d = nc.dram_tensor("B1p", [D], F32, kind="ExternalInput")
    if apply2:
        G2d = nc.dram_tensor("G2p", [D], F32, kind="ExternalInput")
        B2d = nc.dram_tensor("B2p", [D], F32, kind="ExternalInput")
    OUTd = nc.dram_tensor("OUTp", [BL, P, NQT, D], F32, kind="ExternalOutput")

    with tile.TileContext(nc) as tc:
        with (
            tc.tile_pool(name="const", bufs=1) as pc,
            tc.tile_pool(name="wts", bufs=1) as pw,
            tc.tile_pool(name="qkin", bufs=2) as pin,
            tc.tile_pool(name="mid", bufs=1) as pmid,
            tc.tile_pool(name="stream", bufs=2) as pst,
            tc.tile_pool(name="small", bufs=2) as psm,
            tc.tile_pool(name="psS", bufs=3, space="PSUM") as psS,
            tc.tile_pool(name="psT", bufs=2, space="PSUM") as psT,
            tc.tile_pool(name="psU", bufs=3, space="PSUM") as psU,
        ):
            state = {}

            # ---- warmup before anything else: the PE's first instruction
            # must only wait on the tiny bf16 identity load, and alternating
            # PSUM dests keep LDWEIGHTS pipelined (full PE duty for the HAM).
            identb = pc.tile([P, P], BF16, name="identb")
            nc.sync.dma_start(identb, IDBd[:, :])
            wps = psT.tile([P, 4, P], BF16, tag="tr", name="warm")
            for w in range(NWARM):
                nc.tensor.matmul(
                    wps[:, 0 : 2, :].bitcast(F32)[:, w % 2, 0:P//2],
                    identb, identb[:, 0:P//2],
                    start=True, stop=True,
                )

            # ---- constants (tiny, land immediately) ----
            identr = pc.tile([P, P], F32R, name="identr")
            nc.sync.dma_start(identr, IDd[:, :].bitcast(F32R))
            onesc = pc.tile([P, 1], BF16)
            nc.sync.dma_start(onesc, ONEd[:, :])
            epsb = pc.tile([P, 1], F32)
            nc.vector.memset(epsb, EPS)
            b1b = pc.tile([P, 1], F32)
            nc.vector.memset(b1b, b1f)
            W1Ts = pw.tile([P, NDT, D], BF16)
            W2Ts = pw.tile([P, NDT, D], BF16)
            if apply1:
                g1t = pc.tile([P, D], F32)
                nc.gpsimd.dma_start(
                    g1t, bass.AP(tensor=G1d, offset=0, ap=[[0, P], [1, D]])
                )
                b1t = pc.tile([P, D], F32)
                nc.gpsimd.dma_start(
                    b1t, bass.AP(tensor=B1d, offset=0, ap=[[0, P], [1, D]])
                )
            if apply2:
                g2t = pc.tile([P, D], F32)
                nc.gpsimd.dma_start(
                    g2t, bass.AP(tensor=G2d, offset=0, ap=[[0, P], [1, D]])
                )
                b2t = pc.tile([P, D], F32)
                nc.gpsimd.dma_start(
                    b2t, bass.AP(tensor=B2d, offset=0, ap=[[0, P], [1, D]])
                )

            def emit_loads(b):
                # input loads for batch b, all on the sync queue in priority
                # order; batch 0 interleaves per-dt chunks so dt-outer S(0)
                # starts on the first arrivals
                QTs = pin.tile([P, NDT, QTL], BF16, tag="qts")
                KTs = pin.tile([P, NDT, KTL], BF16, tag="kts")
                # batch 0: small 2-dt lead chunk so dt-outer S(0) starts on
                # the first arrival, then one large-descriptor bulk chunk
                chunks = [(0, 2), (2, NDT)] if b == 0 else [(0, 4), (4, NDT)]
                for lo, hi in chunks:
                    sl = slice(lo, hi)
                    nc.sync.dma_start(QTs[:, sl, :], QTd[b][:, sl, :])
                    nc.sync.dma_start(KTs[:, sl, :], KTd[b][:, sl, :])
                kbr = psm.tile([1, KTL], BF16, tag="kbr")
                nc.sync.dma_start(kbr, KBd[b][None, :])
                qmr = psm.tile([1, QTL], BF16, tag="qmr")
                nc.sync.dma_start(qmr, QMd[b][None, :])
                Vs = pin.tile([P, NKT, D], BF16, tag="vs")
                nc.sync.dma_start(Vs, Vd[b][:, :, :])
                qps = pst.tile([P, NQT, D], F32, tag="qn", bufs=2)
                nc.sync.dma_start(qps, Qd[b][:, :, :])
                state[("in", b)] = (QTs, KTs, Vs, qps, kbr, qmr)

            def emit_S(b, yt_cb=None):
                # S^T = K Qs^T + kbias (x) qmask, then exp -> E^T (bf16).
                # kt-outer in steady state (1 PSUM bank live, exp overlaps);
                # dt-outer for batch 0 (4 banks, 2 borrowed from the then-idle
                # psU pool) so the matmuls chase the arriving per-dt chunks.
                QTs, KTs, _, _, kbr, qmr = state[("in", b)]
                ET = pmid.tile([P, NKT, QTL], BF16, tag="eh", bufs=2)
                if b == 0:
                    Sps = [
                        psS.tile([P, QTL], F32, tag="s", name="s0"),
                        psS.tile([P, QTL], F32, tag="s", name="s1"),
                        psU.tile([P, QTL], F32, tag="u", name="s2"),
                        psU.tile([P, QTL], F32, tag="u", name="s3"),
                    ]
                    for dt in range(NDT):
                        for kt in range(NKT):
                            nc.tensor.matmul(
                                Sps[kt],
                                KTs[:, dt, kt * P : (kt + 1) * P],
                                QTs[:, dt, :],
                                start=(dt == 0),
                                stop=False,
                            )
                    for kt in range(NKT):
                        nc.tensor.matmul(
                            Sps[kt],
                            kbr[:, kt * P : (kt + 1) * P],
                            qmr[:, :],
                            start=False,
                            stop=True,
                        )
                        nc.scalar.activation(ET[:, kt, :], Sps[kt], AF.Exp)
                else:
                    # exps emitted as one contiguous ACT block after the
                    # matmuls: exactly one Exp-table load per batch. The
                    # yT transpose groups of the PREVIOUS batch interleave
                    # between the kt groups (via yt_cb) so each one arrives
                    # after its qt's LN1 has cleared the DVE queue.
                    Sl = []
                    for kt in range(NKT):
                        Sp = psS.tile([P, QTL], F32, tag="s")
                        for dt in range(NDT):
                            nc.tensor.matmul(
                                Sp,
                                KTs[:, dt, kt * P : (kt + 1) * P],
                                QTs[:, dt, :],
                                start=(dt == 0),
                                stop=False,
                            )
                        nc.tensor.matmul(
                            Sp,
                            kbr[:, kt * P : (kt + 1) * P],
                            qmr[:, :],
                            start=False,
                            stop=True,
                        )
                        Sl.append(Sp)
                        if yt_cb is not None:
                            yt_cb(kt)
                        if kt >= 1:
                            nc.scalar.activation(
                                ET[:, kt - 1, :], Sl[kt - 1], AF.Exp
                            )
                    nc.scalar.activation(ET[:, NKT - 1, :], Sl[NKT - 1], AF.Exp)
                state[("et", b)] = ET

            def emit_rowsum(b):
                ET = state[("et", b)]
                rs = psS.tile([P, QTL], F32, tag="s", name="rs")
                rsum = rs[:, 0:NQT]
                for qt in range(NQT):
                    for kt in range(NKT):
                        nc.tensor.matmul(
                            rsum[:, qt : qt + 1],
                            ET[:, kt, qt * P : (qt + 1) * P],
                            onesc,
                            start=(kt == 0),
                            stop=(kt == NKT - 1),
                        )
                recip = psm.tile([P, NQT], F32, tag="recip")
                nc.vector.reciprocal(recip, rsum)
                state[("recip", b)] = recip

            def emit_U(b, stagger_yt=False):
                # U = E V scaled by recip, fused residual add, LN1 -> y
                _, _, Vs, qps, _, _ = state[("in", b)]
                ET = state[("et", b)]
                recip = state[("recip", b)]
                y = pmid.tile([P, NQT, D], BF16, tag="y", bufs=2)
                state[("y", b)] = y
                for qt in range(NQT):
                    qres = pst.tile([P, D], F32, tag="big4", bufs=3)
                    for ch in range(NCH):
                        Ups = psU.tile([P, 512], F32, tag="u")
                        for kt in range(NKT):
                            nc.tensor.matmul(
                                Ups,
                                ET[:, kt, qt * P : (qt + 1) * P],
                                Vs[:, kt, ch * 512 : (ch + 1) * 512],
                                start=(kt == 0),
                                stop=(kt == NKT - 1),
                            )
                        qch = qres[:, ch * 512 : (ch + 1) * 512]
                        nc.vector.scalar_tensor_tensor(
                            qch,
                            Ups,
                            recip[:, qt : qt + 1],
                            qps[:, qt, ch * 512 : (ch + 1) * 512],
                            op0=ALU.mult,
                            op1=ALU.add,
                        )
                        if ch == 0:
                            stats = psm.tile([P, NCH, 6], F32, tag="st1")
                        nc.vector.bn_stats(
                            stats[:, ch, :], qres[:, ch * 512 : (ch + 1) * 512]
                        )
                    mv = psm.tile([P, 2], F32, tag="mv1")
                    nc.vector.bn_aggr(mv, stats)
                    std = psm.tile([P, 1], F32, tag="std1")
                    nc.scalar.activation(std, mv[:, 1:2], AF.Sqrt, bias=epsb[:, :])
                    nc.vector.reciprocal(std, std)
                    nc.vector.tensor_scalar(
                        y[:, qt, :],
                        qres,
                        scalar1=mv[:, 0:1],
                        scalar2=std,
                        op0=ALU.subtract,
                        op1=ALU.mult,
                    )
                    if apply1:
                        yf = y[:, qt, :]
                        nc.vector.tensor_mul(yf, yf, g1t)
                        nc.vector.tensor_add(yf, yf, b1t)
                    if stagger_yt and qt >= 2:
                        emit_ytr_qt(b, qt - 2)
                state[("y", b)] = y

            def emit_ytr_qt(b, qt):
                # y^T transposes for one q tile (2 PSUM groups of 4)
                y = state[("y", b)]
                if ("yt", b) not in state:
                    YTn = pmid.tile([P, NDT, QTL], BF16, tag="yt", bufs=2)
                    state[("yt", b)] = YTn
                YT = state[("yt", b)]
                for half in range(2):
                    tps = psT.tile([P, 4, P], BF16, tag="tr", name="tps")
                    for j in range(4):
                        dt = half * 4 + j
                        nc.tensor.transpose(
                            tps[:, j, :],
                            y[:, qt, dt * P : (dt + 1) * P],
                            identb,
                        )
                    # steady batches: route the last qt's copies to DVE --
                    # ACT is the convoy in the S-window while DVE has
                    # end-of-window slack, so the FFN1 gate lands earlier
                    dst = YT[:, half * 4 : half * 4 + 4, qt * P : (qt + 1) * P]
                    if qt == NQT - 1 and b < BL - 1:
                        nc.vector.tensor_copy(dst, tps)
                    else:
                        nc.scalar.copy(dst, tps)

            def emit_ffn1(b, qsl=None, ots=None):
                # qsl: optional (q0, q1) column range for split FFN1 (the last
                # batch runs half-q FFN1 over qt 0-1 while qt 2-3's LN1 and
                # transposes drain); ots: optional subset of output tiles
                YT = state[("yt", b)]
                if ("ht", b) not in state:
                    HTn = pmid.tile([P, NDT, QTL], BF16, tag="ht")
                    state[("ht", b)] = HTn
                HT = state[("ht", b)]
                q0, q1 = (0, QTL) if qsl is None else qsl
                for ot in (range(NDT) if ots is None else ots):
                    Hps = psU.tile([P, QTL], F32, tag="u")
                    for dt in range(NDT):
                        nc.tensor.matmul(
                            Hps[:, q0:q1],
                            W1Ts[:, dt, ot * P : (ot + 1) * P],
                            YT[:, dt, q0:q1],
                            start=(dt == 0),
                            stop=(dt == NDT - 1),
                        )
                    nc.scalar.activation(
                        HT[:, ot, q0:q1], Hps[:, q0:q1], AF.Relu, bias=b1b[:, :]
                    )

            def emit_ffn2(b):
                HT = state[("ht", b)]
                y = state[("y", b)]
                for qt in range(NQT):
                    r2 = pst.tile([P, D], F32, tag="big4", bufs=3)
                    for ch in range(NCH):
                        Zps = psS.tile([P, QTL], F32, tag="s", name="zps")
                        for ot in range(NDT):
                            nc.tensor.matmul(
                                Zps,
                                HT[:, ot, qt * P : (qt + 1) * P],
                                W2Ts[:, ot, ch * 512 : (ch + 1) * 512],
                                start=(ot == 0),
                                stop=(ot == NDT - 1),
                            )
                        nc.vector.tensor_add(
                            r2[:, ch * 512 : (ch + 1) * 512],
                            Zps,
                            y[:, qt, ch * 512 : (ch + 1) * 512],
                        )
                        if ch == 0:
                            stats2 = psm.tile([P, NCH, 6], F32, tag="st2")
                        nc.vector.bn_stats(
                            stats2[:, ch, :], r2[:, ch * 512 : (ch + 1) * 512]
                        )
                    mv2 = psm.tile([P, 2], F32, tag="mv2")
                    nc.vector.bn_aggr(mv2, stats2)
                    std2 = psm.tile([P, 1], F32, tag="std2")
                    nc.scalar.activation(std2, mv2[:, 1:2], AF.Sqrt, bias=epsb[:, :])
                    nc.vector.reciprocal(std2, std2)
                    stg = pst.tile([P, D], F32, tag="stg", bufs=2)
                    for ch in range(NCH):
                        csl = slice(ch * 512, (ch + 1) * 512)
                        nc.vector.tensor_scalar(
                            stg[:, csl],
                            r2[:, csl],
                            scalar1=mv2[:, 0:1],
                            scalar2=std2,
                            op0=ALU.subtract,
                            op1=ALU.mult,
                        )
                        if apply2:
                            nc.vector.tensor_mul(stg[:, csl], stg[:, csl], g2t[:, csl])
                            nc.vector.tensor_add(stg[:, csl], stg[:, csl], b2t[:, csl])
                        seng = nc.scalar if ch == 0 else nc.sync
                        seng.dma_start(OUTd[b][:, qt, csl], stg[:, csl])

            # ---- prologue: batch-0 loads, weights, warmup, S(0) ----
            emit_loads(0)
            if BL > 1:
                emit_loads(1)
            nc.sync.dma_start(W1Ts, W1Td[:, :, :])
            nc.sync.dma_start(W2Ts, W2Td[:, :, :])
            emit_S(0)

            # ---- steady-state pipeline ----
            for b in range(BL):
                last = b + 1 == BL
                emit_rowsum(b)
                emit_U(b, stagger_yt=last)
                if not last:
                    emit_S(b + 1, yt_cb=lambda kt, bb=b: emit_ytr_qt(bb, kt))
                    if b + 2 < BL:
                        emit_loads(b + 2)
                    emit_ffn1(b)
                else:
                    emit_ffn1(b, (0, QTL // 2), ots=range(0, 4))
                    emit_ytr_qt(b, NQT - 2)
                    emit_ffn1(b, (0, QTL // 2), ots=range(4, NDT))
                    emit_ytr_qt(b, NQT - 1)
                    emit_ffn1(b, (QTL // 2, QTL))
                emit_ffn2(b)

    nc.finalize()
    return nc


def _prepare(Q, K, V, Q_lengths, K_lengths, W1, b1, W2, b2,
             ln1_g, ln1_b, ln2_g, ln2_b):
    Q = np.asarray(Q, dtype=np.float32)
    K = np.asarray(K, dtype=np.float32)
    V = np.asarray(V, dtype=np.float32)
    W1 = np.asarray(W1, dtype=np.float32)
    W2 = np.asarray(W2, dtype=np.float32)
    qlen = np.asarray(Q_lengths).astype(np.int64)
    klen = np.asarray(K_lengths).astype(np.int64)
    g1 = np.asarray(ln1_g, dtype=np.float32)
    b1v = np.asarray(ln1_b, dtype=np.float32)
    g2 = np.asarray(ln2_g, dtype=np.float32)
    b2v = np.asarray(ln2_b, dtype=np.float32)
    b1f = float(np.asarray(b1, dtype=np.float32).reshape(-1)[0])
    # b2 cancels exactly inside LN2 (constant shift removed by mean
    # subtraction), so it is not passed to the device.

    apply1 = not (np.all(g1 == 1.0) and np.all(b1v == 0.0))
    apply2 = not (np.all(g2 == 1.0) and np.all(b2v == 0.0))

    qmask = (np.arange(QTL)[None, :] < qlen[:, None]).astype(np.float32)  # [B,QT]
    kmask = np.arange(KTL)[None, :] < klen[:, None]  # [B,KT]

    # Q^T pre-scaled by 1/sqrt(D), invalid-q rows zeroed -> exp(0)=1 there.
    Qs = Q * (qmask / np.sqrt(np.float32(D)))[:, :, None]
    # partition-major swizzles: [b, p, t, free] with the last two dims
    # contiguous per partition for large DMA descriptors
    QT = Qs.transpose(0, 2, 1)  # [B, D, QT]
    QT = np.ascontiguousarray(
        QT.reshape(B, NDT, P, QTL).transpose(0, 2, 1, 3)
    ).astype(BF16NP)
    KT = K.transpose(0, 2, 1)
    KT = np.ascontiguousarray(
        KT.reshape(B, NDT, P, KTL).transpose(0, 2, 1, 3)
    ).astype(BF16NP)
    Vb = np.ascontiguousarray(
        V.reshape(B, NKT, P, D).transpose(0, 2, 1, 3)
    ).astype(BF16NP)
    Qr = np.ascontiguousarray(Q.reshape(B, NQT, P, D).transpose(0, 2, 1, 3))
    W1T = np.ascontiguousarray(
        W1.T.reshape(NDT, P, D).transpose(1, 0, 2)
    ).astype(BF16NP)
    W2T = np.ascontiguousarray(
        W2.T.reshape(NDT, P, D).transpose(1, 0, 2)
    ).astype(BF16NP)

    kb = np.where(kmask, 0.0, NEG).astype(BF16NP)  # [B,KT]
    qm = qmask.astype(BF16NP)  # [B,QT]
    ident = np.eye(P, dtype=np.float32)
    identb = np.eye(P, dtype=np.float32).astype(BF16NP)
    ones = np.ones((P, 1), dtype=BF16NP)

    nc = _build(apply1, apply2, b1f)

    in_maps = []
    for c in range(NCORES):
        s = slice(c * BL, (c + 1) * BL)
        m = {
            "QTp": QT[s],
            "KTp": KT[s],
            "Vp": Vb[s],
            "Qp": Qr[s],
            "W1Tp": W1T,
            "W2Tp": W2T,
            "QMp": qm[s],
            "KBp": kb[s],
            "IDp": ident,
            "IDBp": identb,
            "ONEp": ones,
        }
        if apply1:
            m["G1p"] = g1
            m["B1p"] = b1v
        if apply2:
            m["G2p"] = g2
            m["B2p"] = b2v
        in_maps.append(m)

    return nc, in_maps


def kernel(**inputs):
    nc, in_maps = _prepare(**inputs)
    res = run_bass_kernel_spmd(nc, in_maps, list(range(NCORES)))
    out = np.concatenate([res.results[c]["OUTp"] for c in range(NCORES)], axis=0)
    # un-swizzle [B, P, NQT, D] -> [B, QTL, D] (q = qt*P + p)
    out = out.reshape(B, P, NQT, D).transpose(0, 2, 1, 3).reshape(B, QTL, D)
    return np.ascontiguousarray(out).astype(np.float32)
